# revision 32
# baseline (speedup 1.0000x reference)
"""Trainium2 Bass kernel for AttnDecoderRNN single step.

Sharding: batch-parallel attention+GRU (16 rows/core), all-gather of
transposed features, vocab-parallel output projection (6250 cols/core).
"""

import os
import numpy as np
import ml_dtypes

import concourse.bass as bass
import concourse.tile as tile
from concourse import mybir, bacc, bass_utils
from concourse.masks import make_identity

B, S, V, E, H = 128, 256, 50000, 300, 300
ENC2 = 600
NCORES = 8
BL = B // NCORES          # 16 batch rows per core
VL = V // NCORES          # 6250 vocab cols per core
F = H + ENC2 + E          # 1200 feature dim
FB = F + 1                # +1 bias/ones row
GI_K = E + 1 + ENC2       # 901 (emb, bias-ones, attn)

F32 = mybir.dt.float32
BF16 = mybir.dt.bfloat16
BF = ml_dtypes.bfloat16


def _chunks(total, step=128):
    return [(s, min(step, total - s)) for s in range(0, total, step)]


_CACHE = {}


def _build(single=False):
    nc = bacc.Bacc("TRN2", target_bir_lowering=False, debug=False,
                   num_devices=(1 if single else NCORES))

    # ---- I/O ----
    ctxT_d = nc.dram_tensor("ctxT", [ENC2, BL, S], F32, kind="ExternalInput")
    hT_d = nc.dram_tensor("hT_ext", [H + 1, BL], F32, kind="ExternalInput")
    hid_d = nc.dram_tensor("hid", [BL, H], F32, kind="ExternalInput")
    embT_d = nc.dram_tensor("embT_ext", [E + 1, BL], F32, kind="ExternalInput")
    embTb_d = nc.dram_tensor("embT_ext_bf", [E + 1, BL], BF16, kind="ExternalInput")
    Whe_d = nc.dram_tensor("W_h_ext", [H + 1, H], F32, kind="ExternalInput")
    Wc_d = nc.dram_tensor("W_c", [ENC2, H], BF16, kind="ExternalInput")
    vT_d = nc.dram_tensor("vT", [H, 1], BF16, kind="ExternalInput")
    Wih_d = nc.dram_tensor("W_ihT", [GI_K, 3 * H], F32, kind="ExternalInput")
    Whh_d = nc.dram_tensor("W_hhT", [H + 1, 3 * H], F32, kind="ExternalInput")
    Wout_d = nc.dram_tensor("out_WT", [FB, VL], BF16, kind="ExternalInput")

    logits_o = nc.dram_tensor("logits_o", [B, VL], F32, kind="ExternalOutput")
    hnew_o = nc.dram_tensor("hnew_o", [BL, H], F32, kind="ExternalOutput")
    attnw_o = nc.dram_tensor("attnw_o", [BL, S], F32, kind="ExternalOutput")

    w_dram = nc.dram_tensor("w_dram", [BL, S], BF16)
    hwb_dram = nc.dram_tensor("hwb_dram", [BL, H], BF16)
    cc_in = nc.dram_tensor("cc_in", [FB, BL], BF16)
    cc_out = nc.dram_tensor("cc_out", [NCORES, FB, BL], BF16)

    CH_H = _chunks(H)        # 3 chunks of 300
    CH_E2 = _chunks(ENC2)    # 5 chunks of 600
    CH_H1 = _chunks(H + 1)   # 3 chunks of 301
    CH_FB = _chunks(FB)      # 10 chunks of 1201
    CH_VL = _chunks(VL, 512)  # 13 chunks of 6250

    with tile.TileContext(nc) as tc:
        with (
            tc.tile_pool(name="const", bufs=1) as cp,
            tc.tile_pool(name="ctxf", bufs=2) as ctxf,
            tc.tile_pool(name="et", bufs=2) as etp,
            tc.tile_pool(name="wm", bufs=2) as wmp,
            tc.tile_pool(name="wt", bufs=8) as wtp,
            tc.tile_pool(name="lsp", bufs=2) as lsp,
            tc.tile_pool(name="ps", bufs=2, space="PSUM") as pp,
        ):
            # ---- constant/weight loads ----
            ident16 = cp.tile([16, 16], F32, tag="ident16")
            make_identity(nc, ident16[:, :])
            ones_row = cp.tile([1, S], BF16, tag="ones_row")
            nc.vector.memset(ones_row[:, :], 1.0)

            hT_sb = []
            for i, (ks, ksz) in enumerate(CH_H1):
                t = cp.tile([ksz, BL], F32, tag=f"hT{i}", name=f"hT{i}")
                nc.sync.dma_start(out=t[:, :], in_=hT_d[ks:ks + ksz, :])
                hT_sb.append(t)
            embT_sb = []
            for i, (ks, ksz) in enumerate(CH_H1):
                t = cp.tile([ksz, BL], F32, tag=f"embT{i}", name=f"embT{i}")
                nc.sync.dma_start(out=t[:, :], in_=embT_d[ks:ks + ksz, :])
                embT_sb.append(t)
            Whe_sb = []
            for i, (ks, ksz) in enumerate(CH_H1):
                t = cp.tile([ksz, H], F32, tag=f"Whe{i}", name=f"Whe{i}")
                nc.sync.dma_start(out=t[:, :], in_=Whe_d[ks:ks + ksz, :])
                Whe_sb.append(t)
            Wc_sb = []
            for i, (ks, ksz) in enumerate(CH_E2):
                t = cp.tile([ksz, H], BF16, tag=f"Wc{i}", name=f"Wc{i}")
                nc.sync.dma_start(out=t[:, :], in_=Wc_d[ks:ks + ksz, :])
                Wc_sb.append(t)
            vT_sb = []
            for i, (ks, ksz) in enumerate(CH_H):
                t = cp.tile([ksz, 1], BF16, tag=f"vT{i}", name=f"vT{i}")
                nc.sync.dma_start(out=t[:, :], in_=vT_d[ks:ks + ksz, :])
                vT_sb.append(t)
            # GRU weights: K-tiling of W_ihT rows [0:301]=emb+bias, [301:901]=attn
            gi_rows = [(ks, ksz) for (ks, ksz) in CH_H1] + \
                      [(301 + ds, dsz) for (ds, dsz) in CH_E2]
            Wih_sb = []
            for i, (ks, ksz) in enumerate(gi_rows):
                t = cp.tile([ksz, 3 * H], F32, tag=f"Wih{i}", name=f"Wih{i}")
                nc.sync.dma_start(out=t[:, :], in_=Wih_d[ks:ks + ksz, :])
                Wih_sb.append(t)
            Whh_sb = []
            for i, (ks, ksz) in enumerate(CH_H1):
                t = cp.tile([ksz, 3 * H], F32, tag=f"Whh{i}", name=f"Whh{i}")
                nc.sync.dma_start(out=t[:, :], in_=Whh_d[ks:ks + ksz, :])
                Whh_sb.append(t)
            hid_sb = cp.tile([BL, H], F32, tag="hid_sb")
            nc.sync.dma_start(out=hid_sb[:, :], in_=hid_d[:, :])

            # ---- context load + bf16 cast ----
            ctxbf = []
            for i, (ds, dsz) in enumerate(CH_E2):
                cf = ctxf.tile([dsz, BL, S], F32, tag="cf", name="cf")
                nc.sync.dma_start(out=cf[:, :, :], in_=ctxT_d[ds:ds + dsz, :, :])
                cb = cp.tile([dsz, BL, S], BF16, tag=f"ctxbf{i}",
                             name=f"ctxbf{i}")
                nc.vector.tensor_copy(cb[:, :, :], cf[:, :, :])
                ctxbf.append(cb)

            # ---- hWb = hT_ext.T @ W_h_ext (+attn_b via ones row) ----
            p_hwb = pp.tile([BL, H], F32, tag="g", name="p_hwb")
            for i in range(3):
                nc.tensor.matmul(p_hwb[:, :], lhsT=hT_sb[i][:, :],
                                 rhs=Whe_sb[i][:, :],
                                 start=(i == 0), stop=(i == 2))
            hWb_bf = cp.tile([BL, H], BF16, tag="hWb_bf")
            nc.vector.tensor_copy(hWb_bf[:, :], p_hwb[:, :])
            # flatten hWb rows into partition 0 (matmul lhsT must start at
            # a PE quadrant boundary, so per-row partition slices are out)
            nc.sync.dma_start(out=hwb_dram[:, :], in_=hWb_bf[:, :])
            hwb_flat = cp.tile([1, BL * H], BF16, tag="hwb_flat")
            nc.sync.dma_start(out=hwb_flat[:, :],
                              in_=hwb_dram.ap().rearrange("a b -> (a b)")[None, :])

            # ---- attention scores per batch row ----
            scores_sb = cp.tile([BL, S], F32, tag="scores_sb")
            for b in range(BL):
                ets = []
                for mi, (ms, msz) in enumerate(CH_H):
                    pe = pp.tile([msz, S], F32, tag="e")
                    for di in range(len(CH_E2)):
                        nc.tensor.matmul(
                            pe[:, :],
                            lhsT=Wc_sb[di][:, ms:ms + msz],
                            rhs=ctxbf[di][:, b, :],
                            start=(di == 0), stop=False)
                    nc.tensor.matmul(
                        pe[:, :], lhsT=hwb_flat[:, b * H + ms:b * H + ms + msz],
                        rhs=ones_row[:, :], start=False, stop=True)
                    et = etp.tile([msz, S], BF16, tag=f"et{mi}", name=f"et{mi}")
                    nc.scalar.activation(et[:, :], pe[:, :],
                                         mybir.ActivationFunctionType.Tanh)
                    ets.append(et)
                p_sc = pp.tile([1, S], F32, tag="sc", bufs=1)
                for mi in range(3):
                    nc.tensor.matmul(
                        p_sc[:, :], lhsT=vT_sb[mi][:, :],
                        rhs=ets[mi][:, :], start=(mi == 0), stop=(mi == 2))
                sc_row = etp.tile([1, S], F32, tag="sc_row")
                nc.vector.tensor_copy(sc_row[:, :], p_sc[:, :])
                nc.sync.dma_start(out=scores_sb[b:b + 1, :], in_=sc_row[:, :])

            # ---- softmax over S (mask is all-ones) ----
            maxv = cp.tile([BL, 1], F32, tag="maxv")
            nc.vector.reduce_max(out=maxv[:, :], in_=scores_sb[:, :],
                                 axis=mybir.AxisListType.X)
            negmax = cp.tile([BL, 1], F32, tag="negmax")
            nc.vector.tensor_scalar_mul(negmax[:, :], maxv[:, :], -1.0)
            exp_sb = cp.tile([BL, S], F32, tag="exp_sb")
            sumexp = cp.tile([BL, 1], F32, tag="sumexp")
            nc.scalar.activation(exp_sb[:, :], scores_sb[:, :],
                                 mybir.ActivationFunctionType.Exp,
                                 bias=negmax[:, :], accum_out=sumexp[:, :])
            rsum = cp.tile([BL, 1], F32, tag="rsum")
            nc.vector.reciprocal(rsum[:, :], sumexp[:, :])
            attnw_sb = cp.tile([BL, S], F32, tag="attnw_sb")
            nc.vector.tensor_scalar_mul(attnw_sb[:, :], exp_sb[:, :], rsum[:, :])
            nc.sync.dma_start(out=attnw_o[:, :], in_=attnw_sb[:, :])
            w_bf = cp.tile([BL, S], BF16, tag="w_bf")
            nc.vector.tensor_scalar_mul(w_bf[:, :], exp_sb[:, :], rsum[:, :])
            nc.sync.dma_start(out=w_dram[:, :], in_=w_bf[:, :])

            # broadcast weights to all 128 partitions
            w_bc = cp.tile([128, BL, S], BF16, tag="w_bc")
            wd_ap = w_dram[:, :]
            bcast = bass.AP(tensor=wd_ap.tensor, offset=wd_ap.offset,
                            ap=[[0, 128]] + list(wd_ap.ap))
            nc.sync.dma_start(out=w_bc[:, :, :], in_=bcast)

            # ---- weighted context sum -> attnT [d, b] (DVE) ----
            attnT = []
            attnT_bf = []
            for i, (ds, dsz) in enumerate(CH_E2):
                wm = wmp.tile([dsz, BL, S], BF16, tag="wm", name="wm")
                nc.vector.tensor_mul(wm[:, :, :], ctxbf[i][:, :, :],
                                     w_bc[:dsz, :, :])
                at = cp.tile([dsz, BL], F32, tag=f"attnT{i}", name=f"attnT{i}")
                nc.vector.reduce_sum(out=at[:, :], in_=wm[:, :, :],
                                     axis=mybir.AxisListType.X)
                attnT.append(at)
                ab = cp.tile([dsz, BL], BF16, tag=f"attnTb{i}", name=f"attnTb{i}")
                nc.vector.tensor_copy(ab[:, :], at[:, :])
                attnT_bf.append(ab)
                nc.sync.dma_start(out=cc_in[H + ds:H + ds + dsz, :],
                                  in_=ab[:, :])

            # ---- GRU cell (f32) ----
            gi_lhs = embT_sb + attnT
            p_gi = []
            for g in range(3):
                pg = pp.tile([BL, H], F32, tag="g", name=f"pgi{g}")
                for i in range(len(gi_lhs)):
                    nc.tensor.matmul(
                        pg[:, :], lhsT=gi_lhs[i][:, :],
                        rhs=Wih_sb[i][:, g * H:(g + 1) * H],
                        start=(i == 0), stop=(i == len(gi_lhs) - 1))
                sg = cp.tile([BL, H], F32, tag=f"gi_sb{g}", name=f"gi_sb{g}")
                nc.vector.tensor_copy(sg[:, :], pg[:, :])
                p_gi.append(sg)
            p_gh = []
            for g in range(3):
                pg = pp.tile([BL, H], F32, tag="g", name=f"pgh{g}")
                for i in range(3):
                    nc.tensor.matmul(
                        pg[:, :], lhsT=hT_sb[i][:, :],
                        rhs=Whh_sb[i][:, g * H:(g + 1) * H],
                        start=(i == 0), stop=(i == 2))
                sg = cp.tile([BL, H], F32, tag=f"gh_sb{g}", name=f"gh_sb{g}")
                nc.vector.tensor_copy(sg[:, :], pg[:, :])
                p_gh.append(sg)

            r_t = cp.tile([BL, H], F32, tag="r_t")
            nc.vector.tensor_add(r_t[:, :], p_gi[0][:, :], p_gh[0][:, :])
            r_sb = cp.tile([BL, H], F32, tag="r_sb")
            nc.scalar.activation(r_sb[:, :], r_t[:, :],
                                 mybir.ActivationFunctionType.Sigmoid)
            z_t = cp.tile([BL, H], F32, tag="z_t")
            nc.vector.tensor_add(z_t[:, :], p_gi[1][:, :], p_gh[1][:, :])
            z_sb = cp.tile([BL, H], F32, tag="z_sb")
            nc.scalar.activation(z_sb[:, :], z_t[:, :],
                                 mybir.ActivationFunctionType.Sigmoid)
            n_t = cp.tile([BL, H], F32, tag="n_t")
            nc.vector.tensor_mul(n_t[:, :], r_sb[:, :], p_gh[2][:, :])
            n_t2 = cp.tile([BL, H], F32, tag="n_t2")
            nc.vector.tensor_add(n_t2[:, :], n_t[:, :], p_gi[2][:, :])
            n_sb = cp.tile([BL, H], F32, tag="n_sb")
            nc.scalar.activation(n_sb[:, :], n_t2[:, :],
                                 mybir.ActivationFunctionType.Tanh)
            d_t = cp.tile([BL, H], F32, tag="d_t")
            nc.vector.tensor_sub(d_t[:, :], hid_sb[:, :], n_sb[:, :])
            d_t2 = cp.tile([BL, H], F32, tag="d_t2")
            nc.vector.tensor_mul(d_t2[:, :], z_sb[:, :], d_t[:, :])
            hnew_sb = cp.tile([BL, H], F32, tag="hnew_sb")
            nc.vector.tensor_add(hnew_sb[:, :], n_sb[:, :], d_t2[:, :])
            nc.sync.dma_start(out=hnew_o[:, :], in_=hnew_sb[:, :])

            # ---- build cc_in = [h_newT; attnT; embT; ones] (bf16) ----
            for i, (ms, msz) in enumerate(CH_H):
                pt = pp.tile([msz, BL], F32, tag="t", bufs=1, name=f"pt{i}")
                nc.tensor.transpose(pt[:, :], hnew_sb[:, ms:ms + msz],
                                    ident16[:, :])
                hb = cp.tile([msz, BL], BF16, tag=f"hnT{i}", name=f"hnT{i}")
                nc.vector.tensor_copy(hb[:, :], pt[:, :])
                nc.sync.dma_start(out=cc_in[ms:ms + msz, :], in_=hb[:, :])
            nc.sync.dma_start(out=cc_in[H + ENC2:FB, :], in_=embTb_d[:, :])

            # ---- all-gather features across cores ----
            if single:
                for c in range(NCORES):
                    nc.sync.dma_start(out=cc_out[c, :, :], in_=cc_in[:, :])
            else:
                nc.gpsimd.collective_compute(
                    "AllGather", mybir.AluOpType.bypass,
                    replica_groups=[list(range(NCORES))],
                    ins=[cc_in.ap().opt()], outs=[cc_out.ap().opt()])

            # ---- gathered features -> SBUF lhsT tiles ----
            gath = cc_out.ap().rearrange("c d b -> d c b")
            ft_sb = []
            for i, (ks, ksz) in enumerate(CH_FB):
                t = cp.tile([ksz, B], BF16, tag=f"ft{i}", name=f"ft{i}")
                nc.sync.dma_start(
                    out=t[:, :].rearrange("p (c b) -> p c b", c=NCORES),
                    in_=gath[ks:ks + ksz, :, :])
                ft_sb.append(t)

            # ---- vocab-sharded output projection ----
            for ni, (ns, nsz) in enumerate(CH_VL):
                pl = pp.tile([B, nsz], F32, tag="l")
                for ki, (ks, ksz) in enumerate(CH_FB):
                    wt = wtp.tile([ksz, nsz], BF16, tag="wt", name="wt")
                    nc.sync.dma_start(out=wt[:, :],
                                      in_=Wout_d[ks:ks + ksz, ns:ns + nsz])
                    nc.tensor.matmul(pl[:, :], lhsT=ft_sb[ki][:, :],
                                     rhs=wt[:, :], start=(ki == 0),
                                     stop=(ki == len(CH_FB) - 1))
                ls = lsp.tile([B, nsz], F32, tag="ls", name="ls")
                nc.vector.tensor_copy(ls[:, :], pl[:, :])
                nc.sync.dma_start(out=logits_o[:, ns:ns + nsz], in_=ls[:, :])

    nc.compile()
    return nc


def kernel(inputs, hidden, context, mask, emb_table, attn_W, attn_b, attn_v,
           W_ih, b_ih, W_hh, b_hh, out_W, out_b):
    inputs = np.asarray(inputs)
    hidden = np.asarray(hidden, dtype=np.float32)
    context = np.asarray(context, dtype=np.float32)
    emb_table = np.asarray(emb_table, dtype=np.float32)
    attn_W = np.asarray(attn_W, dtype=np.float32)
    attn_b = np.asarray(attn_b, dtype=np.float32)
    attn_v = np.asarray(attn_v, dtype=np.float32)
    W_ih = np.asarray(W_ih, dtype=np.float32)
    b_ih = np.asarray(b_ih, dtype=np.float32)
    W_hh = np.asarray(W_hh, dtype=np.float32)
    b_hh = np.asarray(b_hh, dtype=np.float32)
    out_W = np.asarray(out_W, dtype=np.float32)
    out_b = np.asarray(out_b, dtype=np.float32)

    if "nc" not in _CACHE:
        _CACHE["nc"] = _build()
    nc = _CACHE["nc"]

    embedded = emb_table[inputs[:, 0].astype(np.int64)]       # [B, E]
    ones_b = np.ones((1, BL), np.float32)

    # shared weights
    Whe = np.concatenate([attn_W[:H, :], attn_b[None, :]], 0)           # [301,300]
    Wc = np.ascontiguousarray(attn_W[H:, :]).astype(BF)                 # [600,300]
    vT = attn_v[:, None].astype(BF)                                     # [300,1]
    WihT = W_ih.T                                                       # [900,900]
    Wih_ext = np.concatenate([WihT[:E], b_ih[None, :], WihT[E:]], 0)    # [901,900]
    Whh_ext = np.concatenate([W_hh.T, b_hh[None, :]], 0)                # [301,900]
    WoutT = np.concatenate([out_W.T, out_b[None, :]], 0).astype(BF)     # [1201,V]

    in_maps = []
    for c in range(NCORES):
        sh = slice(c * BL, (c + 1) * BL)
        ctxT_c = np.ascontiguousarray(context[sh].transpose(2, 0, 1))
        hT_ext = np.concatenate([hidden[sh].T, ones_b], 0)
        embT_ext = np.concatenate([embedded[sh].T, ones_b], 0)
        in_maps.append({
            "ctxT": ctxT_c,
            "hT_ext": np.ascontiguousarray(hT_ext),
            "hid": np.ascontiguousarray(hidden[sh]),
            "embT_ext": np.ascontiguousarray(embT_ext),
            "embT_ext_bf": np.ascontiguousarray(embT_ext).astype(BF),
            "W_h_ext": Whe,
            "W_c": Wc,
            "vT": vT,
            "W_ihT": np.ascontiguousarray(Wih_ext),
            "W_hhT": np.ascontiguousarray(Whh_ext),
            "out_WT": np.ascontiguousarray(WoutT[:, c * VL:(c + 1) * VL]),
        })

    _CACHE["in_maps"] = in_maps
    res = bass_utils.run_bass_kernel_spmd(
        nc, in_maps, core_ids=list(range(NCORES)),
        trace=bool(int(os.environ.get("KERNEL_TRACE", "0"))))
    _CACHE["last_results"] = res

    logits = np.concatenate([res.results[c]["logits_o"] for c in range(NCORES)],
                            axis=1)
    h_new = np.concatenate([res.results[c]["hnew_o"] for c in range(NCORES)],
                           axis=0)
    attn_w = np.concatenate([res.results[c]["attnw_o"] for c in range(NCORES)],
                            axis=0)
    return logits, h_new, attn_w


# revision 44
# speedup vs baseline: 1.0602x; 1.0602x over previous
"""Trainium2 Bass kernel for AttnDecoderRNN single step.

Sharding: batch-parallel attention+GRU (16 rows/core), all-gather of
transposed features, vocab-parallel output projection (6250 cols/core).
"""

import os
import numpy as np
import ml_dtypes

import concourse.bass as bass
import concourse.tile as tile
from concourse import mybir, bacc, bass_utils
from concourse.masks import make_identity

B, S, V, E, H = 128, 256, 50000, 300, 300
ENC2 = 600
NCORES = 8
BL = B // NCORES          # 16 batch rows per core
VL = V // NCORES          # 6250 vocab cols per core
F = H + ENC2 + E          # 1200 feature dim
FB = F + 1                # +1 bias/ones row
GI_K = E + 1 + ENC2       # 901 (emb, bias-ones, attn)

F32 = mybir.dt.float32
BF16 = mybir.dt.bfloat16
BF = ml_dtypes.bfloat16


def _chunks(total, step=128):
    return [(s, min(step, total - s)) for s in range(0, total, step)]


_CACHE = {}


def _build(single=False):
    nc = bacc.Bacc("TRN2", target_bir_lowering=False, debug=False,
                   num_devices=(1 if single else NCORES))

    # ---- I/O ----
    ctxT_d = nc.dram_tensor("ctxT", [ENC2, BL, S], F32, kind="ExternalInput")
    hT_d = nc.dram_tensor("hT_ext", [H + 1, BL], F32, kind="ExternalInput")
    hid_d = nc.dram_tensor("hid", [BL, H], F32, kind="ExternalInput")
    embT_d = nc.dram_tensor("embT_ext", [E + 1, BL], F32, kind="ExternalInput")
    embTb_d = nc.dram_tensor("embT_ext_bf", [E + 1, BL], BF16, kind="ExternalInput")
    Whe_d = nc.dram_tensor("W_h_ext", [H + 1, H], F32, kind="ExternalInput")
    Wc_d = nc.dram_tensor("W_c", [ENC2, H], BF16, kind="ExternalInput")
    vT_d = nc.dram_tensor("vT", [H, 1], BF16, kind="ExternalInput")
    Wih_d = nc.dram_tensor("W_ihT", [GI_K, 3 * H], F32, kind="ExternalInput")
    Whh_d = nc.dram_tensor("W_hhT", [H + 1, 3 * H], F32, kind="ExternalInput")
    Wout_d = nc.dram_tensor("out_WT", [FB, VL], BF16, kind="ExternalInput")
    ones2_d = nc.dram_tensor("ones2", [2, 2 * S], BF16, kind="ExternalInput")

    logits_o = nc.dram_tensor("logits_o", [B, VL], F32, kind="ExternalOutput")
    hnew_o = nc.dram_tensor("hnew_o", [BL, H], F32, kind="ExternalOutput")
    attnw_o = nc.dram_tensor("attnw_o", [BL, S], F32, kind="ExternalOutput")

    w_dram = nc.dram_tensor("w_dram", [BL, S], BF16)
    hwb_dram = nc.dram_tensor("hwb_dram", [BL, H], BF16)
    cc_in = nc.dram_tensor("cc_in", [FB, BL], BF16)
    cc_out = nc.dram_tensor("cc_out", [NCORES, FB, BL], BF16)

    CH_H = _chunks(H)        # 3 chunks of 300
    CH_E2 = _chunks(ENC2)    # 5 chunks of 600
    CH_H1 = _chunks(H + 1)   # 3 chunks of 301
    CH_FB = _chunks(FB)      # 10 chunks of 1201
    CH_VL = _chunks(VL, 512)  # 13 chunks of 6250

    with tile.TileContext(nc) as tc:
        with (
            tc.tile_pool(name="const", bufs=1) as cp,
            tc.tile_pool(name="ctxf", bufs=2) as ctxf,
            tc.tile_pool(name="et", bufs=3) as etp,
            tc.tile_pool(name="wm", bufs=1) as wmp,
            tc.tile_pool(name="wt", bufs=20) as wtp,
            tc.tile_pool(name="lsp", bufs=2) as lsp,
            tc.tile_pool(name="ps", bufs=2, space="PSUM") as pp,
        ):
            # ---- constant/weight loads ----
            ident16 = cp.tile([16, 16], F32, tag="ident16")
            make_identity(nc, ident16[:, :])
            ones2 = cp.tile([2, 2, S], BF16, tag="ones2")
            nc.sync.dma_start(
                out=ones2[:, :, :],
                in_=ones2_d.ap().rearrange("k (b s) -> k b s", b=2))

            hT_sb = []
            for i, (ks, ksz) in enumerate(CH_H1):
                t = cp.tile([ksz, BL], F32, tag=f"hT{i}", name=f"hT{i}")
                nc.sync.dma_start(out=t[:, :], in_=hT_d[ks:ks + ksz, :])
                hT_sb.append(t)
            embT_sb = []
            for i, (ks, ksz) in enumerate(CH_H1):
                t = cp.tile([ksz, BL], F32, tag=f"embT{i}", name=f"embT{i}")
                nc.sync.dma_start(out=t[:, :], in_=embT_d[ks:ks + ksz, :])
                embT_sb.append(t)
            Whe_sb = []
            for i, (ks, ksz) in enumerate(CH_H1):
                t = cp.tile([ksz, H], F32, tag=f"Whe{i}", name=f"Whe{i}")
                nc.sync.dma_start(out=t[:, :], in_=Whe_d[ks:ks + ksz, :])
                Whe_sb.append(t)
            Wc_sb = []
            for i, (ks, ksz) in enumerate(CH_E2):
                t = cp.tile([ksz, H], BF16, tag=f"Wc{i}", name=f"Wc{i}")
                nc.sync.dma_start(out=t[:, :], in_=Wc_d[ks:ks + ksz, :])
                Wc_sb.append(t)
            vT_sb = []
            for i, (ks, ksz) in enumerate(CH_H):
                t = cp.tile([ksz, 1], BF16, tag=f"vT{i}", name=f"vT{i}")
                nc.sync.dma_start(out=t[:, :], in_=vT_d[ks:ks + ksz, :])
                vT_sb.append(t)
            # GRU weights: K-tiling of W_ihT rows [0:301]=emb+bias, [301:901]=attn
            gi_rows = [(ks, ksz) for (ks, ksz) in CH_H1] + \
                      [(301 + ds, dsz) for (ds, dsz) in CH_E2]
            Wih_sb = []
            for i, (ks, ksz) in enumerate(gi_rows):
                t = cp.tile([ksz, 3 * H], F32, tag=f"Wih{i}", name=f"Wih{i}")
                nc.sync.dma_start(out=t[:, :], in_=Wih_d[ks:ks + ksz, :])
                Wih_sb.append(t)
            Whh_sb = []
            for i, (ks, ksz) in enumerate(CH_H1):
                t = cp.tile([ksz, 3 * H], F32, tag=f"Whh{i}", name=f"Whh{i}")
                nc.sync.dma_start(out=t[:, :], in_=Whh_d[ks:ks + ksz, :])
                Whh_sb.append(t)
            hid_sb = cp.tile([BL, H], F32, tag="hid_sb")
            nc.sync.dma_start(out=hid_sb[:, :], in_=hid_d[:, :])

            # ---- context load + bf16 cast ----
            ctxbf = []
            for i, (ds, dsz) in enumerate(CH_E2):
                cf = ctxf.tile([dsz, BL, S], F32, tag="cf", name="cf")
                nc.sync.dma_start(out=cf[:, :, :], in_=ctxT_d[ds:ds + dsz, :, :])
                cb = cp.tile([dsz, BL, S], BF16, tag=f"ctxbf{i}",
                             name=f"ctxbf{i}")
                nc.vector.tensor_copy(cb[:, :, :], cf[:, :, :])
                ctxbf.append(cb)

            # ---- hWb = hT_ext.T @ W_h_ext (+attn_b via ones row) ----
            p_hwb = pp.tile([BL, H], F32, tag="g", name="p_hwb")
            for i in range(3):
                nc.tensor.matmul(p_hwb[:, :], lhsT=hT_sb[i][:, :],
                                 rhs=Whe_sb[i][:, :],
                                 start=(i == 0), stop=(i == 2))
            hWb_bf = cp.tile([BL, H], BF16, tag="hWb_bf")
            nc.vector.tensor_copy(hWb_bf[:, :], p_hwb[:, :])
            # flatten hWb rows into partition 0 (matmul lhsT must start at
            # a PE quadrant boundary, so per-row partition slices are out)
            nc.sync.dma_start(out=hwb_dram[:, :], in_=hWb_bf[:, :])
            hwb2 = cp.tile([2, BL // 2, H], BF16, tag="hwb2")
            nc.sync.dma_start(
                out=hwb2[:, :, :],
                in_=hwb_dram.ap().rearrange("(p two) h -> two p h", two=2))

            p_gh = []
            for g in range(3):
                pg = pp.tile([BL, H], F32, tag="g", name=f"pgh{g}")
                for i in range(3):
                    nc.tensor.matmul(
                        pg[:, :], lhsT=hT_sb[i][:, :],
                        rhs=Whh_sb[i][:, g * H:(g + 1) * H],
                        start=(i == 0), stop=(i == 2))
                sg = cp.tile([BL, H], F32, tag=f"gh_sb{g}", name=f"gh_sb{g}")
                nc.vector.tensor_copy(sg[:, :], pg[:, :])
                p_gh.append(sg)

            # ---- attention scores, two batch rows per matmul ----
            scores_sb = cp.tile([BL, S], F32, tag="scores_sb")
            for pr in range(BL // 2):
                ets = []
                for mi, (ms, msz) in enumerate(CH_H):
                    pe = pp.tile([msz, 2, S], F32, tag="e")
                    for di in range(len(CH_E2)):
                        nc.tensor.matmul(
                            pe[:, :, :],
                            lhsT=Wc_sb[di][:, ms:ms + msz],
                            rhs=ctxbf[di][:, 2 * pr:2 * pr + 2, :],
                            start=(di == 0), stop=False)
                    nc.tensor.matmul(
                        pe[:, :, :], lhsT=hwb2[:, pr, ms:ms + msz],
                        rhs=ones2[:, :, :], start=False, stop=True)
                    et = etp.tile([msz, 2, S], BF16, tag=f"et{mi}",
                                  name=f"et{mi}")
                    nc.scalar.activation(et[:, :, :], pe[:, :, :],
                                         mybir.ActivationFunctionType.Tanh)
                    ets.append(et)
                p_sc = pp.tile([1, 2, S], F32, tag="sc", bufs=1)
                for mi in range(3):
                    nc.tensor.matmul(
                        p_sc[:, :, :], lhsT=vT_sb[mi][:, :],
                        rhs=ets[mi][:, :, :], start=(mi == 0), stop=(mi == 2))
                sc_row = etp.tile([1, 2, S], F32, tag="sc_row")
                nc.vector.tensor_copy(sc_row[:, :, :], p_sc[:, :, :])
                for b2 in range(2):
                    nc.sync.dma_start(
                        out=scores_sb[2 * pr + b2:2 * pr + b2 + 1, :],
                        in_=sc_row[0:1, b2, :])

            # ---- softmax over S (mask is all-ones) ----
            maxv = cp.tile([BL, 1], F32, tag="maxv")
            nc.vector.reduce_max(out=maxv[:, :], in_=scores_sb[:, :],
                                 axis=mybir.AxisListType.X)
            negmax = cp.tile([BL, 1], F32, tag="negmax")
            nc.vector.tensor_scalar_mul(negmax[:, :], maxv[:, :], -1.0)
            exp_sb = cp.tile([BL, S], F32, tag="exp_sb")
            sumexp = cp.tile([BL, 1], F32, tag="sumexp")
            nc.scalar.activation(exp_sb[:, :], scores_sb[:, :],
                                 mybir.ActivationFunctionType.Exp,
                                 bias=negmax[:, :], accum_out=sumexp[:, :])
            rsum = cp.tile([BL, 1], F32, tag="rsum")
            nc.vector.reciprocal(rsum[:, :], sumexp[:, :])
            attnw_sb = cp.tile([BL, S], F32, tag="attnw_sb")
            nc.vector.tensor_scalar_mul(attnw_sb[:, :], exp_sb[:, :], rsum[:, :])
            nc.sync.dma_start(out=attnw_o[:, :], in_=attnw_sb[:, :])
            w_bf = cp.tile([BL, S], BF16, tag="w_bf")
            nc.vector.tensor_scalar_mul(w_bf[:, :], exp_sb[:, :], rsum[:, :])
            nc.sync.dma_start(out=w_dram[:, :], in_=w_bf[:, :])

            # broadcast weights to all 128 partitions
            w_bc = cp.tile([128, BL, S], BF16, tag="w_bc")
            wd_ap = w_dram[:, :]
            bcast = bass.AP(tensor=wd_ap.tensor, offset=wd_ap.offset,
                            ap=[[0, 128]] + list(wd_ap.ap))
            nc.sync.dma_start(out=w_bc[:, :, :], in_=bcast)

            # ---- weighted context sum -> attnT [d, b] (DVE) ----
            attnT = []
            attnT_bf = []
            for i, (ds, dsz) in enumerate(CH_E2):
                wm = wmp.tile([dsz, BL, S], BF16, tag="wm", name="wm")
                nc.vector.tensor_mul(wm[:, :, :], ctxbf[i][:, :, :],
                                     w_bc[:dsz, :, :])
                at = cp.tile([dsz, BL], F32, tag=f"attnT{i}", name=f"attnT{i}")
                nc.vector.reduce_sum(out=at[:, :], in_=wm[:, :, :],
                                     axis=mybir.AxisListType.X)
                attnT.append(at)
                ab = cp.tile([dsz, BL], BF16, tag=f"attnTb{i}", name=f"attnTb{i}")
                nc.vector.tensor_copy(ab[:, :], at[:, :])
                attnT_bf.append(ab)
                nc.sync.dma_start(out=cc_in[H + ds:H + ds + dsz, :],
                                  in_=ab[:, :])

            # ---- GRU cell (f32) ----
            gi_lhs = embT_sb + attnT
            p_gi = []
            for g in range(3):
                pg = pp.tile([BL, H], F32, tag="g", name=f"pgi{g}")
                for i in range(len(gi_lhs)):
                    nc.tensor.matmul(
                        pg[:, :], lhsT=gi_lhs[i][:, :],
                        rhs=Wih_sb[i][:, g * H:(g + 1) * H],
                        start=(i == 0), stop=(i == len(gi_lhs) - 1))
                sg = cp.tile([BL, H], F32, tag=f"gi_sb{g}", name=f"gi_sb{g}")
                nc.vector.tensor_copy(sg[:, :], pg[:, :])
                p_gi.append(sg)

            r_t = cp.tile([BL, H], F32, tag="r_t")
            nc.vector.tensor_add(r_t[:, :], p_gi[0][:, :], p_gh[0][:, :])
            r_sb = cp.tile([BL, H], F32, tag="r_sb")
            nc.scalar.activation(r_sb[:, :], r_t[:, :],
                                 mybir.ActivationFunctionType.Sigmoid)
            z_t = cp.tile([BL, H], F32, tag="z_t")
            nc.vector.tensor_add(z_t[:, :], p_gi[1][:, :], p_gh[1][:, :])
            z_sb = cp.tile([BL, H], F32, tag="z_sb")
            nc.scalar.activation(z_sb[:, :], z_t[:, :],
                                 mybir.ActivationFunctionType.Sigmoid)
            n_t = cp.tile([BL, H], F32, tag="n_t")
            nc.vector.tensor_mul(n_t[:, :], r_sb[:, :], p_gh[2][:, :])
            n_t2 = cp.tile([BL, H], F32, tag="n_t2")
            nc.vector.tensor_add(n_t2[:, :], n_t[:, :], p_gi[2][:, :])
            n_sb = cp.tile([BL, H], F32, tag="n_sb")
            nc.scalar.activation(n_sb[:, :], n_t2[:, :],
                                 mybir.ActivationFunctionType.Tanh)
            d_t = cp.tile([BL, H], F32, tag="d_t")
            nc.vector.tensor_sub(d_t[:, :], hid_sb[:, :], n_sb[:, :])
            d_t2 = cp.tile([BL, H], F32, tag="d_t2")
            nc.vector.tensor_mul(d_t2[:, :], z_sb[:, :], d_t[:, :])
            hnew_sb = cp.tile([BL, H], F32, tag="hnew_sb")
            nc.vector.tensor_add(hnew_sb[:, :], n_sb[:, :], d_t2[:, :])
            nc.sync.dma_start(out=hnew_o[:, :], in_=hnew_sb[:, :])

            # ---- build cc_in = [h_newT; attnT; embT; ones] (bf16) ----
            for i, (ms, msz) in enumerate(CH_H):
                pt = pp.tile([msz, BL], F32, tag="t", bufs=1, name=f"pt{i}")
                nc.tensor.transpose(pt[:, :], hnew_sb[:, ms:ms + msz],
                                    ident16[:, :])
                hb = cp.tile([msz, BL], BF16, tag=f"hnT{i}", name=f"hnT{i}")
                nc.vector.tensor_copy(hb[:, :], pt[:, :])
                nc.sync.dma_start(out=cc_in[ms:ms + msz, :], in_=hb[:, :])
            nc.sync.dma_start(out=cc_in[H + ENC2:FB, :], in_=embTb_d[:, :])

            # ---- all-gather features across cores ----
            if single:
                for c in range(NCORES):
                    nc.sync.dma_start(out=cc_out[c, :, :], in_=cc_in[:, :])
            else:
                nc.gpsimd.collective_compute(
                    "AllGather", mybir.AluOpType.bypass,
                    replica_groups=[list(range(NCORES))],
                    ins=[cc_in.ap().opt()], outs=[cc_out.ap().opt()])

            # ---- gathered features -> SBUF lhsT tiles ----
            gath = cc_out.ap().rearrange("c d b -> d c b")
            ft_sb = []
            for i, (ks, ksz) in enumerate(CH_FB):
                t = cp.tile([ksz, B], BF16, tag=f"ft{i}", name=f"ft{i}")
                nc.sync.dma_start(
                    out=t[:, :].rearrange("p (c b) -> p c b", c=NCORES),
                    in_=gath[ks:ks + ksz, :, :])
                ft_sb.append(t)

            # ---- vocab-sharded output projection ----
            for ni, (ns, nsz) in enumerate(CH_VL):
                pl = pp.tile([B, nsz], F32, tag="l")
                for ki, (ks, ksz) in enumerate(CH_FB):
                    wt = wtp.tile([ksz, nsz], BF16, tag="wt", name="wt")
                    eng = nc.scalar if (ni * len(CH_FB) + ki) % 2 == 0 \
                        else nc.sync
                    eng.dma_start(out=wt[:, :],
                                  in_=Wout_d[ks:ks + ksz, ns:ns + nsz])
                    nc.tensor.matmul(pl[:, :], lhsT=ft_sb[ki][:, :],
                                     rhs=wt[:, :], start=(ki == 0),
                                     stop=(ki == len(CH_FB) - 1))
                ls = lsp.tile([B, nsz], F32, tag="ls", name="ls")
                nc.vector.tensor_copy(ls[:, :], pl[:, :])
                nc.scalar.dma_start(out=logits_o[:, ns:ns + nsz], in_=ls[:, :])

    nc.compile()
    return nc


def kernel(inputs, hidden, context, mask, emb_table, attn_W, attn_b, attn_v,
           W_ih, b_ih, W_hh, b_hh, out_W, out_b):
    inputs = np.asarray(inputs)
    hidden = np.asarray(hidden, dtype=np.float32)
    context = np.asarray(context, dtype=np.float32)
    emb_table = np.asarray(emb_table, dtype=np.float32)
    attn_W = np.asarray(attn_W, dtype=np.float32)
    attn_b = np.asarray(attn_b, dtype=np.float32)
    attn_v = np.asarray(attn_v, dtype=np.float32)
    W_ih = np.asarray(W_ih, dtype=np.float32)
    b_ih = np.asarray(b_ih, dtype=np.float32)
    W_hh = np.asarray(W_hh, dtype=np.float32)
    b_hh = np.asarray(b_hh, dtype=np.float32)
    out_W = np.asarray(out_W, dtype=np.float32)
    out_b = np.asarray(out_b, dtype=np.float32)

    if "nc" not in _CACHE:
        _CACHE["nc"] = _build()
    nc = _CACHE["nc"]

    embedded = emb_table[inputs[:, 0].astype(np.int64)]       # [B, E]
    ones_b = np.ones((1, BL), np.float32)

    # shared weights
    Whe = np.concatenate([attn_W[:H, :], attn_b[None, :]], 0)           # [301,300]
    Wc = np.ascontiguousarray(attn_W[H:, :]).astype(BF)                 # [600,300]
    vT = attn_v[:, None].astype(BF)                                     # [300,1]
    WihT = W_ih.T                                                       # [900,900]
    Wih_ext = np.concatenate([WihT[:E], b_ih[None, :], WihT[E:]], 0)    # [901,900]
    Whh_ext = np.concatenate([W_hh.T, b_hh[None, :]], 0)                # [301,900]
    WoutT = np.concatenate([out_W.T, out_b[None, :]], 0).astype(BF)     # [1201,V]

    in_maps = []
    for c in range(NCORES):
        sh = slice(c * BL, (c + 1) * BL)
        ctxT_c = np.ascontiguousarray(context[sh].transpose(2, 0, 1))
        hT_ext = np.concatenate([hidden[sh].T, ones_b], 0)
        embT_ext = np.concatenate([embedded[sh].T, ones_b], 0)
        in_maps.append({
            "ctxT": ctxT_c,
            "ones2": np.kron(np.eye(2, dtype=np.float32),
                             np.ones((1, S), np.float32)).astype(BF),
            "hT_ext": np.ascontiguousarray(hT_ext),
            "hid": np.ascontiguousarray(hidden[sh]),
            "embT_ext": np.ascontiguousarray(embT_ext),
            "embT_ext_bf": np.ascontiguousarray(embT_ext).astype(BF),
            "W_h_ext": Whe,
            "W_c": Wc,
            "vT": vT,
            "W_ihT": np.ascontiguousarray(Wih_ext),
            "W_hhT": np.ascontiguousarray(Whh_ext),
            "out_WT": np.ascontiguousarray(WoutT[:, c * VL:(c + 1) * VL]),
        })

    _CACHE["in_maps"] = in_maps
    res = bass_utils.run_bass_kernel_spmd(
        nc, in_maps, core_ids=list(range(NCORES)),
        trace=bool(int(os.environ.get("KERNEL_TRACE", "0"))))
    _CACHE["last_results"] = res

    logits = np.concatenate([res.results[c]["logits_o"] for c in range(NCORES)],
                            axis=1)
    h_new = np.concatenate([res.results[c]["hnew_o"] for c in range(NCORES)],
                           axis=0)
    attn_w = np.concatenate([res.results[c]["attnw_o"] for c in range(NCORES)],
                            axis=0)
    return logits, h_new, attn_w


# revision 51
# speedup vs baseline: 1.1967x; 1.1288x over previous
"""Trainium2 Bass kernel for AttnDecoderRNN single step.

Sharding: batch-parallel attention+GRU (16 rows/core), all-gather of
transposed features, vocab-parallel output projection (6250 cols/core).
"""

import os
import numpy as np
import ml_dtypes

import concourse.bass as bass
import concourse.tile as tile
from concourse import mybir, bacc, bass_utils
from concourse.masks import make_identity

B, S, V, E, H = 128, 256, 50000, 300, 300
ENC2 = 600
NCORES = 8
BL = B // NCORES          # 16 batch rows per core
VL = V // NCORES          # 6250 vocab cols per core
F = H + ENC2 + E          # 1200 feature dim
FB = F + 1                # +1 bias/ones row
GI_K = E + 1 + ENC2       # 901 (emb, bias-ones, attn)

F32 = mybir.dt.float32
BF16 = mybir.dt.bfloat16
BF = ml_dtypes.bfloat16


def _chunks(total, step=128):
    return [(s, min(step, total - s)) for s in range(0, total, step)]


_CACHE = {}


def _build(single=False):
    nc = bacc.Bacc("TRN2", target_bir_lowering=False, debug=False,
                   num_devices=(1 if single else NCORES))

    # ---- I/O ----
    ctxT_d = nc.dram_tensor("ctxT", [ENC2, BL, S], BF16, kind="ExternalInput")
    hT_d = nc.dram_tensor("hT_ext", [H + 1, BL], F32, kind="ExternalInput")
    hid_d = nc.dram_tensor("hid", [BL, H], F32, kind="ExternalInput")
    embT_d = nc.dram_tensor("embT_ext", [E + 1, BL], F32, kind="ExternalInput")
    embTb_d = nc.dram_tensor("embT_ext_bf", [E + 1, BL], BF16, kind="ExternalInput")
    Whe_d = nc.dram_tensor("W_h_ext", [H + 1, H], F32, kind="ExternalInput")
    Wc_d = nc.dram_tensor("W_c", [ENC2, H], BF16, kind="ExternalInput")
    vT_d = nc.dram_tensor("vT", [H, 1], BF16, kind="ExternalInput")
    Wih_d = nc.dram_tensor("W_ihT", [GI_K, 3 * H], F32, kind="ExternalInput")
    Whh_d = nc.dram_tensor("W_hhT", [H + 1, 3 * H], F32, kind="ExternalInput")
    Wout_d = nc.dram_tensor("out_WT", [FB, VL], BF16, kind="ExternalInput")
    ones2_d = nc.dram_tensor("ones2", [2, 2 * S], BF16, kind="ExternalInput")

    logits_o = nc.dram_tensor("logits_o", [B, VL], F32, kind="ExternalOutput")
    hnew_o = nc.dram_tensor("hnew_o", [BL, H], F32, kind="ExternalOutput")
    attnw_o = nc.dram_tensor("attnw_o", [BL, S], F32, kind="ExternalOutput")

    w_dram = nc.dram_tensor("w_dram", [BL, S], BF16)
    hwb_dram = nc.dram_tensor("hwb_dram", [BL, H], BF16)
    cc_in = nc.dram_tensor("cc_in", [FB, BL], BF16)
    cc_out = nc.dram_tensor("cc_out", [NCORES, FB, BL], BF16)

    CH_H = _chunks(H)        # 3 chunks of 300
    CH_E2 = _chunks(ENC2)    # 5 chunks of 600
    CH_H1 = _chunks(H + 1)   # 3 chunks of 301
    CH_FB = _chunks(FB)      # 10 chunks of 1201
    CH_VL = _chunks(VL, 512)  # 13 chunks of 6250

    with tile.TileContext(nc) as tc:
        with (
            tc.tile_pool(name="const", bufs=1) as cp,
            tc.tile_pool(name="et", bufs=3) as etp,
            tc.tile_pool(name="wm", bufs=1) as wmp,
            tc.tile_pool(name="wt", bufs=52) as wtp,
            tc.tile_pool(name="lsp", bufs=2) as lsp,
            tc.tile_pool(name="ps", bufs=2, space="PSUM") as pp,
        ):
            # ---- constant/weight loads ----
            ident16 = cp.tile([16, 16], F32, tag="ident16")
            make_identity(nc, ident16[:, :])
            ones2 = cp.tile([2, 2, S], BF16, tag="ones2")
            nc.sync.dma_start(
                out=ones2[:, :, :],
                in_=ones2_d.ap().rearrange("k (b s) -> k b s", b=2))

            hT_sb = []
            for i, (ks, ksz) in enumerate(CH_H1):
                t = cp.tile([ksz, BL], F32, tag=f"hT{i}", name=f"hT{i}")
                nc.sync.dma_start(out=t[:, :], in_=hT_d[ks:ks + ksz, :])
                hT_sb.append(t)
            embT_sb = []
            for i, (ks, ksz) in enumerate(CH_H1):
                t = cp.tile([ksz, BL], F32, tag=f"embT{i}", name=f"embT{i}")
                nc.sync.dma_start(out=t[:, :], in_=embT_d[ks:ks + ksz, :])
                embT_sb.append(t)
            Whe_sb = []
            for i, (ks, ksz) in enumerate(CH_H1):
                t = cp.tile([ksz, H], F32, tag=f"Whe{i}", name=f"Whe{i}")
                nc.sync.dma_start(out=t[:, :], in_=Whe_d[ks:ks + ksz, :])
                Whe_sb.append(t)
            Wc_sb = []
            for i, (ks, ksz) in enumerate(CH_E2):
                t = cp.tile([ksz, H], BF16, tag=f"Wc{i}", name=f"Wc{i}")
                nc.sync.dma_start(out=t[:, :], in_=Wc_d[ks:ks + ksz, :])
                Wc_sb.append(t)
            vT_sb = []
            for i, (ks, ksz) in enumerate(CH_H):
                t = cp.tile([ksz, 1], BF16, tag=f"vT{i}", name=f"vT{i}")
                nc.sync.dma_start(out=t[:, :], in_=vT_d[ks:ks + ksz, :])
                vT_sb.append(t)
            # GRU weights: K-tiling of W_ihT rows [0:301]=emb+bias, [301:901]=attn
            gi_rows = [(ks, ksz) for (ks, ksz) in CH_H1] + \
                      [(301 + ds, dsz) for (ds, dsz) in CH_E2]
            Wih_sb = []
            for i, (ks, ksz) in enumerate(gi_rows):
                t = cp.tile([ksz, 3 * H], F32, tag=f"Wih{i}", name=f"Wih{i}")
                nc.sync.dma_start(out=t[:, :], in_=Wih_d[ks:ks + ksz, :])
                Wih_sb.append(t)
            Whh_sb = []
            for i, (ks, ksz) in enumerate(CH_H1):
                t = cp.tile([ksz, 3 * H], F32, tag=f"Whh{i}", name=f"Whh{i}")
                nc.sync.dma_start(out=t[:, :], in_=Whh_d[ks:ks + ksz, :])
                Whh_sb.append(t)
            hid_sb = cp.tile([BL, H], F32, tag="hid_sb")
            nc.sync.dma_start(out=hid_sb[:, :], in_=hid_d[:, :])

            # ---- context (host-cast bf16, pre-transposed) ----
            ctxbf = []
            for i, (ds, dsz) in enumerate(CH_E2):
                cb = cp.tile([dsz, BL, S], BF16, tag=f"ctxbf{i}",
                             name=f"ctxbf{i}")
                nc.sync.dma_start(out=cb[:, :, :], in_=ctxT_d[ds:ds + dsz, :, :])
                ctxbf.append(cb)

            # ---- hWb = hT_ext.T @ W_h_ext (+attn_b via ones row) ----
            p_hwb = pp.tile([BL, H], F32, tag="g", name="p_hwb")
            for i in range(3):
                nc.tensor.matmul(p_hwb[:, :], lhsT=hT_sb[i][:, :],
                                 rhs=Whe_sb[i][:, :],
                                 start=(i == 0), stop=(i == 2))
            hWb_bf = cp.tile([BL, H], BF16, tag="hWb_bf")
            nc.vector.tensor_copy(hWb_bf[:, :], p_hwb[:, :])
            # flatten hWb rows into partition 0 (matmul lhsT must start at
            # a PE quadrant boundary, so per-row partition slices are out)
            nc.sync.dma_start(out=hwb_dram[:, :], in_=hWb_bf[:, :])
            hwb2 = cp.tile([2, BL // 2, H], BF16, tag="hwb2")
            nc.sync.dma_start(
                out=hwb2[:, :, :],
                in_=hwb_dram.ap().rearrange("(p two) h -> two p h", two=2))

            p_gh = []
            for g in range(3):
                pg = pp.tile([BL, H], F32, tag="g", name=f"pgh{g}")
                for i in range(3):
                    nc.tensor.matmul(
                        pg[:, :], lhsT=hT_sb[i][:, :],
                        rhs=Whh_sb[i][:, g * H:(g + 1) * H],
                        start=(i == 0), stop=(i == 2))
                sg = cp.tile([BL, H], F32, tag=f"gh_sb{g}", name=f"gh_sb{g}")
                nc.vector.tensor_copy(sg[:, :], pg[:, :])
                p_gh.append(sg)

            # ---- attention scores, two batch rows per matmul ----
            scores_sb = cp.tile([BL, S], F32, tag="scores_sb")
            for pr in range(BL // 2):
                ets = []
                for mi, (ms, msz) in enumerate(CH_H):
                    pe = pp.tile([msz, 2, S], F32, tag="e")
                    for di in range(len(CH_E2)):
                        nc.tensor.matmul(
                            pe[:, :, :],
                            lhsT=Wc_sb[di][:, ms:ms + msz],
                            rhs=ctxbf[di][:, 2 * pr:2 * pr + 2, :],
                            start=(di == 0), stop=False)
                    nc.tensor.matmul(
                        pe[:, :, :], lhsT=hwb2[:, pr, ms:ms + msz],
                        rhs=ones2[:, :, :], start=False, stop=True)
                    et = etp.tile([msz, 2, S], BF16, tag=f"et{mi}",
                                  name=f"et{mi}")
                    nc.scalar.activation(et[:, :, :], pe[:, :, :],
                                         mybir.ActivationFunctionType.Tanh)
                    ets.append(et)
                p_sc = pp.tile([1, 2, S], F32, tag="sc", bufs=1)
                for mi in range(3):
                    nc.tensor.matmul(
                        p_sc[:, :, :], lhsT=vT_sb[mi][:, :],
                        rhs=ets[mi][:, :, :], start=(mi == 0), stop=(mi == 2))
                sc_row = etp.tile([1, 2, S], F32, tag="sc_row")
                nc.vector.tensor_copy(sc_row[:, :, :], p_sc[:, :, :])
                for b2 in range(2):
                    nc.sync.dma_start(
                        out=scores_sb[2 * pr + b2:2 * pr + b2 + 1, :],
                        in_=sc_row[0:1, b2, :])

            # ---- softmax over S (mask is all-ones) ----
            maxv = cp.tile([BL, 1], F32, tag="maxv")
            nc.vector.reduce_max(out=maxv[:, :], in_=scores_sb[:, :],
                                 axis=mybir.AxisListType.X)
            negmax = cp.tile([BL, 1], F32, tag="negmax")
            nc.vector.tensor_scalar_mul(negmax[:, :], maxv[:, :], -1.0)
            exp_sb = cp.tile([BL, S], F32, tag="exp_sb")
            sumexp = cp.tile([BL, 1], F32, tag="sumexp")
            nc.scalar.activation(exp_sb[:, :], scores_sb[:, :],
                                 mybir.ActivationFunctionType.Exp,
                                 bias=negmax[:, :], accum_out=sumexp[:, :])
            rsum = cp.tile([BL, 1], F32, tag="rsum")
            nc.vector.reciprocal(rsum[:, :], sumexp[:, :])
            attnw_sb = cp.tile([BL, S], F32, tag="attnw_sb")
            nc.vector.tensor_scalar_mul(attnw_sb[:, :], exp_sb[:, :], rsum[:, :])
            nc.sync.dma_start(out=attnw_o[:, :], in_=attnw_sb[:, :])
            w_bf = cp.tile([BL, S], BF16, tag="w_bf")
            nc.vector.tensor_scalar_mul(w_bf[:, :], exp_sb[:, :], rsum[:, :])
            nc.sync.dma_start(out=w_dram[:, :], in_=w_bf[:, :])

            # broadcast weights to all 128 partitions
            w_bc = cp.tile([128, BL, S], BF16, tag="w_bc")
            wd_ap = w_dram[:, :]
            bcast = bass.AP(tensor=wd_ap.tensor, offset=wd_ap.offset,
                            ap=[[0, 128]] + list(wd_ap.ap))
            nc.sync.dma_start(out=w_bc[:, :, :], in_=bcast)

            # ---- weighted context sum -> attnT [d, b] (DVE) ----
            attnT = []
            attnT_bf = []
            for i, (ds, dsz) in enumerate(CH_E2):
                wm = wmp.tile([dsz, BL, S], BF16, tag="wm", name="wm")
                nc.vector.tensor_mul(wm[:, :, :], ctxbf[i][:, :, :],
                                     w_bc[:dsz, :, :])
                at = cp.tile([dsz, BL], F32, tag=f"attnT{i}", name=f"attnT{i}")
                nc.vector.reduce_sum(out=at[:, :], in_=wm[:, :, :],
                                     axis=mybir.AxisListType.X)
                attnT.append(at)
                ab = cp.tile([dsz, BL], BF16, tag=f"attnTb{i}", name=f"attnTb{i}")
                nc.vector.tensor_copy(ab[:, :], at[:, :])
                attnT_bf.append(ab)
                nc.sync.dma_start(out=cc_in[H + ds:H + ds + dsz, :],
                                  in_=ab[:, :])

            # ---- GRU cell (f32) ----
            gi_lhs = embT_sb + attnT
            p_gi = []
            for g in range(3):
                pg = pp.tile([BL, H], F32, tag="g", name=f"pgi{g}")
                for i in range(len(gi_lhs)):
                    nc.tensor.matmul(
                        pg[:, :], lhsT=gi_lhs[i][:, :],
                        rhs=Wih_sb[i][:, g * H:(g + 1) * H],
                        start=(i == 0), stop=(i == len(gi_lhs) - 1))
                sg = cp.tile([BL, H], F32, tag=f"gi_sb{g}", name=f"gi_sb{g}")
                nc.vector.tensor_copy(sg[:, :], pg[:, :])
                p_gi.append(sg)

            r_t = cp.tile([BL, H], F32, tag="r_t")
            nc.vector.tensor_add(r_t[:, :], p_gi[0][:, :], p_gh[0][:, :])
            r_sb = cp.tile([BL, H], F32, tag="r_sb")
            nc.scalar.activation(r_sb[:, :], r_t[:, :],
                                 mybir.ActivationFunctionType.Sigmoid)
            z_t = cp.tile([BL, H], F32, tag="z_t")
            nc.vector.tensor_add(z_t[:, :], p_gi[1][:, :], p_gh[1][:, :])
            z_sb = cp.tile([BL, H], F32, tag="z_sb")
            nc.scalar.activation(z_sb[:, :], z_t[:, :],
                                 mybir.ActivationFunctionType.Sigmoid)
            n_t = cp.tile([BL, H], F32, tag="n_t")
            nc.vector.tensor_mul(n_t[:, :], r_sb[:, :], p_gh[2][:, :])
            n_t2 = cp.tile([BL, H], F32, tag="n_t2")
            nc.vector.tensor_add(n_t2[:, :], n_t[:, :], p_gi[2][:, :])
            n_sb = cp.tile([BL, H], F32, tag="n_sb")
            nc.scalar.activation(n_sb[:, :], n_t2[:, :],
                                 mybir.ActivationFunctionType.Tanh)
            d_t = cp.tile([BL, H], F32, tag="d_t")
            nc.vector.tensor_sub(d_t[:, :], hid_sb[:, :], n_sb[:, :])
            d_t2 = cp.tile([BL, H], F32, tag="d_t2")
            nc.vector.tensor_mul(d_t2[:, :], z_sb[:, :], d_t[:, :])
            hnew_sb = cp.tile([BL, H], F32, tag="hnew_sb")
            nc.vector.tensor_add(hnew_sb[:, :], n_sb[:, :], d_t2[:, :])
            nc.sync.dma_start(out=hnew_o[:, :], in_=hnew_sb[:, :])

            # ---- build cc_in = [h_newT; attnT; embT; ones] (bf16) ----
            for i, (ms, msz) in enumerate(CH_H):
                pt = pp.tile([msz, BL], F32, tag="t", bufs=1, name=f"pt{i}")
                nc.tensor.transpose(pt[:, :], hnew_sb[:, ms:ms + msz],
                                    ident16[:, :])
                hb = cp.tile([msz, BL], BF16, tag=f"hnT{i}", name=f"hnT{i}")
                nc.vector.tensor_copy(hb[:, :], pt[:, :])
                nc.sync.dma_start(out=cc_in[ms:ms + msz, :], in_=hb[:, :])
            nc.sync.dma_start(out=cc_in[H + ENC2:FB, :], in_=embTb_d[:, :])

            # ---- all-gather features across cores ----
            if single:
                for c in range(NCORES):
                    nc.sync.dma_start(out=cc_out[c, :, :], in_=cc_in[:, :])
            else:
                nc.gpsimd.collective_compute(
                    "AllGather", mybir.AluOpType.bypass,
                    replica_groups=[list(range(NCORES))],
                    ins=[cc_in.ap().opt()], outs=[cc_out.ap().opt()])

            # ---- gathered features -> SBUF lhsT tiles ----
            gath = cc_out.ap().rearrange("c d b -> d c b")
            ft_sb = []
            for i, (ks, ksz) in enumerate(CH_FB):
                t = cp.tile([ksz, B], BF16, tag=f"ft{i}", name=f"ft{i}")
                nc.sync.dma_start(
                    out=t[:, :].rearrange("p (c b) -> p c b", c=NCORES),
                    in_=gath[ks:ks + ksz, :, :])
                ft_sb.append(t)

            # ---- vocab-sharded output projection ----
            for ni, (ns, nsz) in enumerate(CH_VL):
                pl = pp.tile([B, nsz], F32, tag="l")
                for ki, (ks, ksz) in enumerate(CH_FB):
                    wt = wtp.tile([ksz, nsz], BF16, tag="wt", name="wt")
                    eng = nc.scalar if (ni * len(CH_FB) + ki) % 2 == 0 \
                        else nc.sync
                    eng.dma_start(out=wt[:, :],
                                  in_=Wout_d[ks:ks + ksz, ns:ns + nsz])
                    nc.tensor.matmul(pl[:, :], lhsT=ft_sb[ki][:, :],
                                     rhs=wt[:, :], start=(ki == 0),
                                     stop=(ki == len(CH_FB) - 1))
                ls = lsp.tile([B, nsz], F32, tag="ls", name="ls")
                nc.vector.tensor_copy(ls[:, :], pl[:, :])
                nc.scalar.dma_start(out=logits_o[:, ns:ns + nsz], in_=ls[:, :])

    nc.compile()
    return nc


def kernel(inputs, hidden, context, mask, emb_table, attn_W, attn_b, attn_v,
           W_ih, b_ih, W_hh, b_hh, out_W, out_b):
    inputs = np.asarray(inputs)
    hidden = np.asarray(hidden, dtype=np.float32)
    context = np.asarray(context, dtype=np.float32)
    emb_table = np.asarray(emb_table, dtype=np.float32)
    attn_W = np.asarray(attn_W, dtype=np.float32)
    attn_b = np.asarray(attn_b, dtype=np.float32)
    attn_v = np.asarray(attn_v, dtype=np.float32)
    W_ih = np.asarray(W_ih, dtype=np.float32)
    b_ih = np.asarray(b_ih, dtype=np.float32)
    W_hh = np.asarray(W_hh, dtype=np.float32)
    b_hh = np.asarray(b_hh, dtype=np.float32)
    out_W = np.asarray(out_W, dtype=np.float32)
    out_b = np.asarray(out_b, dtype=np.float32)

    if "nc" not in _CACHE:
        _CACHE["nc"] = _build()
    nc = _CACHE["nc"]

    embedded = emb_table[inputs[:, 0].astype(np.int64)]       # [B, E]
    ones_b = np.ones((1, BL), np.float32)

    # shared weights
    Whe = np.concatenate([attn_W[:H, :], attn_b[None, :]], 0)           # [301,300]
    Wc = np.ascontiguousarray(attn_W[H:, :]).astype(BF)                 # [600,300]
    vT = attn_v[:, None].astype(BF)                                     # [300,1]
    WihT = W_ih.T                                                       # [900,900]
    Wih_ext = np.concatenate([WihT[:E], b_ih[None, :], WihT[E:]], 0)    # [901,900]
    Whh_ext = np.concatenate([W_hh.T, b_hh[None, :]], 0)                # [301,900]
    WoutT = np.concatenate([out_W.T, out_b[None, :]], 0).astype(BF)     # [1201,V]

    in_maps = []
    for c in range(NCORES):
        sh = slice(c * BL, (c + 1) * BL)
        ctxT_c = np.ascontiguousarray(context[sh].transpose(2, 0, 1)).astype(BF)
        hT_ext = np.concatenate([hidden[sh].T, ones_b], 0)
        embT_ext = np.concatenate([embedded[sh].T, ones_b], 0)
        in_maps.append({
            "ctxT": ctxT_c,
            "ones2": np.kron(np.eye(2, dtype=np.float32),
                             np.ones((1, S), np.float32)).astype(BF),
            "hT_ext": np.ascontiguousarray(hT_ext),
            "hid": np.ascontiguousarray(hidden[sh]),
            "embT_ext": np.ascontiguousarray(embT_ext),
            "embT_ext_bf": np.ascontiguousarray(embT_ext).astype(BF),
            "W_h_ext": Whe,
            "W_c": Wc,
            "vT": vT,
            "W_ihT": np.ascontiguousarray(Wih_ext),
            "W_hhT": np.ascontiguousarray(Whh_ext),
            "out_WT": np.ascontiguousarray(WoutT[:, c * VL:(c + 1) * VL]),
        })

    _CACHE["in_maps"] = in_maps
    res = bass_utils.run_bass_kernel_spmd(
        nc, in_maps, core_ids=list(range(NCORES)),
        trace=bool(int(os.environ.get("KERNEL_TRACE", "0"))))
    _CACHE["last_results"] = res

    logits = np.concatenate([res.results[c]["logits_o"] for c in range(NCORES)],
                            axis=1)
    h_new = np.concatenate([res.results[c]["hnew_o"] for c in range(NCORES)],
                           axis=0)
    attn_w = np.concatenate([res.results[c]["attnw_o"] for c in range(NCORES)],
                            axis=0)
    return logits, h_new, attn_w
